# revision 54
# baseline (speedup 1.0000x reference)
"""Trainium2 Bass kernel for nn_AdapterBlock (cross-attention adapter block).

Reference (per batch b):
    x   = concat(h[b], vis[b])                    # [S=3072, C=768]
    q   = h @ Wq.T ; k = x @ Wk.T ; v = x @ Wv.T  # H=8 heads, D=96
    att = softmax(q k^T / sqrt(D)) v              # [L, C]
    out = LayerNorm(att @ Wo.T) * gamma + beta

Sharding: 8 cores = (batch b = core//2) x (head-group hg = core%2).
Each core projects K/V/Q only for its own 4 heads (so the pair shares the
projection work -- no duplication), attends over the full L=2048 query
rows for those heads, then the pair exchanges attention outputs via an
AllGather (pairs replica groups) so each core can out-project + LayerNorm
its own half of the query rows with the full 768-dim contraction.

All cross-core asymmetry is resolved host-side so the SPMD program is
static:
  * the h-token rows are rolled per core so "rows this core keeps" are
    always local rows 0..1023 (l-tiles 0-7) and "rows the peer keeps" are
    local l-tiles 8-15 (kv-token order doesn't matter for attention);
  * the gathered buffer has one chunk per pair rank; the out-projection
    contracts over own ao + both gathered chunks, with the Wo rows for
    this core's own chunk zeroed host-side (its data covers the wrong
    rows) and the peer chunk carrying the peer heads' Wo rows.

Orientation choices (the cost model charges a matmul out_free x steps):
  * Q/K projected direct-transposed (qT/kT [96, L/S]) for attention.
  * V natural [s, 97/head] with a ones column => softmax denominator for
    free in the PV matmul's 97th output column.
  * scores [s-tile 128, l 1024]: one matmul per s-tile; exp on ACT.
  * PV natural [l-tile 128, 97] accumulating over 24 s-tiles (lhsT = exp
    tile slice, rhs = v tile): full 128-partition packing.
  * ao -> aoT via DMA xbar transposes (no PE time).
  * O natural [l-tile, 769] with a host row-sums column = LN mean free.
Q/K projections for head h+1 and the V projections are interleaved into
the ACT(exp)-bound attention items with an EDF scheduler. The v
PSUM->SBUF drains run on DVE (not ACT) so the exp stream never waits
behind them, and the xT / weight loads are batched into one strided DMA
per wave (HWDGE issue is ~625ns per instruction and serializes the
prologue).
"""

from contextlib import ExitStack

import numpy as np

import concourse.bacc as bacc
import concourse.mybir as mybir
import concourse.tile as tile
from concourse.bass_utils import run_bass_kernel_spmd

F32 = mybir.dt.float32
BF16 = mybir.dt.bfloat16
AF = mybir.ActivationFunctionType

B, L, NV, C = 4, 2048, 1024, 768
H, D = 8, 96
S = L + NV             # 3072 kv tokens
SCALE = float(D) ** -0.5
N_CORES = 8
NST = S // 128         # 24 s-tiles
EPS = 1e-5
HG = 4                 # heads per core
CG = HG * D            # 384 feature cols per core
VW = 97                # v cols per head (96 + denominator ones col)
LLOC = L // 2          # 1024 query rows kept per core
REP_GROUPS = [[0, 1], [2, 3], [4, 5], [6, 7]]


def build_nc():
    nc = bacc.Bacc("TRN2", target_bir_lowering=False, num_devices=N_CORES)
    xt_in = nc.declare_dram_parameter("xT", [C, S], BF16, isOutput=False)
    wq_in = nc.declare_dram_parameter("wq", [C, CG], BF16, isOutput=False)
    wk_in = nc.declare_dram_parameter("wk", [C, CG], BF16, isOutput=False)
    wv_in = nc.declare_dram_parameter("wv", [C, HG * VW], BF16, isOutput=False)
    # wo rows: [own-ao 384 | gathered-chunk0 384 | gathered-chunk1 384]
    wo_in = nc.declare_dram_parameter("wo", [3 * CG, C + 1], BF16,
                                      isOutput=False)
    gmb_in = nc.declare_dram_parameter("gammab", [128, C], F32, isOutput=False)
    btb_in = nc.declare_dram_parameter("betab", [128, C], F32, isOutput=False)
    out_ext = nc.declare_dram_parameter("out", [LLOC, C], BF16, isOutput=True)

    with tile.TileContext(nc) as tc, ExitStack() as ctx:
        # ---------------- persistent tensors ----------------
        xT_pool = ctx.enter_context(tc.tile_pool(name="xT", bufs=1))
        xT_all = xT_pool.tile([128, 6 * S], BF16, tag="xT", name="xT")
        xT = [xT_all[:].rearrange("p (c s) -> p c s", c=6)[:, ci, :]
              for ci in range(6)]
        w_pool = ctx.enter_context(tc.tile_pool(name="w", bufs=1))
        wq_all = w_pool.tile([128, 6 * CG], BF16, tag="wq", name="wq")
        wk_all = w_pool.tile([128, 6 * CG], BF16, tag="wk", name="wk")
        wv_all = w_pool.tile([128, 6 * HG * VW], BF16, tag="wv", name="wv")
        wq_t = [wq_all[:, CG * ci:CG * (ci + 1)] for ci in range(6)]
        wk_t = [wk_all[:, CG * ci:CG * (ci + 1)] for ci in range(6)]
        wv_t = [wv_all[:, HG * VW * ci:HG * VW * (ci + 1)]
                for ci in range(6)]

        kq_pool = ctx.enter_context(tc.tile_pool(name="kq", bufs=1))
        kT = [kq_pool.tile([96, S], BF16, tag=f"kT{h}", name=f"kT{h}")
              for h in range(HG)]
        qT = [kq_pool.tile([96, L], BF16, tag=f"qT{h}", name=f"qT{h}")
              for h in range(HG)]
        v_pool = ctx.enter_context(tc.tile_pool(name="v", bufs=1))
        v_t = [v_pool.tile([128, HG * VW], BF16, tag=f"v{t}", name=f"v{t}")
               for t in range(NST)]
        ao_pool = ctx.enter_context(tc.tile_pool(name="ao", bufs=1))
        # own 4 heads' normalized attention out, all 16 l-tiles, 384 cols
        ao_t = [ao_pool.tile([128, CG], BF16, tag=f"ao{lt}", name=f"ao{lt}")
                for lt in range(16)]

        const_pool = ctx.enter_context(tc.tile_pool(name="cst", bufs=1))
        gamma_t = const_pool.tile([128, C], F32, tag="gamma")
        beta_t = const_pool.tile([128, C], F32, tag="beta")

        # stream xT in batched strided waves: cols 0:1024 first (covers
        # the prologue's K0/Q0/V needs); the rest follows in two waves.
        xt_src = xt_in[:].rearrange("(c p) s -> p c s", p=128)
        xT_v = xT_all[:].rearrange("p (c s) -> p c s", c=6)
        wk_v = wk_all[:].rearrange("p (c g) -> p c g", c=6)
        wk_src = wk_in[:].rearrange("(c p) g -> p c g", p=128)
        wq_v = wq_all[:].rearrange("p (c g) -> p c g", c=6)
        wq_src = wq_in[:].rearrange("(c p) g -> p c g", p=128)
        nc.sync.dma_start(wk_v[:, 0:3], wk_src[:, 0:3])
        nc.sync.dma_start(xT_v[:, 0:3, 0:512], xt_src[:, 0:3, 0:512])
        nc.sync.dma_start(wk_v[:, 3:6], wk_src[:, 3:6])
        nc.sync.dma_start(xT_v[:, 3:6, 0:512], xt_src[:, 3:6, 0:512])
        nc.sync.dma_start(wq_v[:, 0:3], wq_src[:, 0:3])
        nc.sync.dma_start(xT_v[:, 0:3, 512:1024], xt_src[:, 0:3, 512:1024])
        nc.sync.dma_start(wq_v[:, 3:6], wq_src[:, 3:6])
        nc.sync.dma_start(xT_v[:, 3:6, 512:1024], xt_src[:, 3:6, 512:1024])
        nc.sync.dma_start(wv_all[:].rearrange("p (c g) -> p c g", c=6),
                          wv_in[:].rearrange("(c p) g -> p c g", p=128))
        nc.sync.dma_start(xT_v[:, :, 1024:2048], xt_src[:, :, 1024:2048])
        nc.sync.dma_start(xT_v[:, :, 2048:S], xt_src[:, :, 2048:S])
        nc.sync.dma_start(gamma_t[:], gmb_in[:])
        nc.sync.dma_start(beta_t[:], btb_in[:])
        # Wo is persistent and loaded early so the epilogue never waits on it
        wo_pool = ctx.enter_context(tc.tile_pool(name="wo", bufs=1))
        wo_t = [wo_pool.tile([128, C + 1], BF16, tag=f"wo{k}", name=f"wo{k}")
                for k in range(9)]
        for k in range(9):
            nc.sync.dma_start(wo_t[k][:], wo_in[128 * k:128 * (k + 1), :])

        # exchange buffers: per-head AllGather (pairs), pipelined so each
        # gather overlaps the remaining attention items
        dram = ctx.enter_context(tc.tile_pool(name="dram", bufs=1,
                                              space="DRAM"))
        ag_in = [dram.tile([128, 8 * D], BF16, tag=f"agi{hh}",
                           name=f"agi{hh}") for hh in range(HG)]
        ag_out = [dram.tile([256, 8 * D], BF16, tag=f"ago{hh}",
                            name=f"ago{hh}") for hh in range(HG)]
        recv_pool = ctx.enter_context(tc.tile_pool(name="recv", bufs=1))
        recv = [recv_pool.tile([128, 8 * CG], BF16, tag=f"rc{r}",
                               name=f"rc{r}") for r in range(2)]

        def exchange_head(hh):
            for lt in range(8):
                nc.gpsimd.dma_start(ag_in[hh][:, D * lt:D * (lt + 1)],
                                    ao_t[8 + lt][:, D * hh:D * (hh + 1)])
            nc.gpsimd.collective_compute(
                "AllGather", mybir.AluOpType.bypass,
                replica_groups=REP_GROUPS,
                ins=[ag_in[hh][:]], outs=[ag_out[hh][:]])
            for r in range(2):
                nc.sync.dma_start(
                    recv[r][:].rearrange("p (l c) -> p l c", c=CG)
                    [:, :, D * hh:D * (hh + 1)],
                    ag_out[hh][128 * r:128 * (r + 1), :]
                    .rearrange("p (l c) -> p l c", c=D))

        # ---------------- projection step helpers ----------------
        attn_ctx = ExitStack()
        proj_ps = attn_ctx.enter_context(
            tc.tile_pool(name="proj_ps", bufs=2, space="PSUM"))
        st_state = {}

        def q_step(h, jc, ci):
            key = ("q", h, jc)
            if ci == 0:
                st_state[key] = proj_ps.tile([128, 512], F32, tag="proj",
                                             name="prps")
            t = st_state[key]
            nc.tensor.matmul(
                t[0:96, :], wq_t[ci][:, D * h:D * (h + 1)],
                xT[ci][:, 512 * jc:512 * (jc + 1)],
                start=(ci == 0), stop=(ci == 5))
            if ci == 5:
                nc.vector.tensor_copy(
                    qT[h][:, 512 * jc:512 * (jc + 1)], t[0:96, :])
                del st_state[key]

        def k_step(h, sc, ci):
            key = ("k", h, sc)
            if ci == 0:
                st_state[key] = proj_ps.tile([128, 512], F32, tag="proj",
                                             name="prps")
            t = st_state[key]
            nc.tensor.matmul(
                t[0:96, :], wk_t[ci][:, D * h:D * (h + 1)],
                xT[ci][:, 512 * sc:512 * (sc + 1)],
                start=(ci == 0), stop=(ci == 5))
            if ci == 5:
                nc.vector.tensor_copy(
                    kT[h][:, 512 * sc:512 * (sc + 1)], t[0:96, :])
                del st_state[key]

        def v_step(st, ci):
            key = ("v", st)
            if ci == 0:
                st_state[key] = proj_ps.tile([128, 512], F32, tag="proj",
                                             name="prps")
            t = st_state[key]
            nc.tensor.matmul(
                t[:, 0:HG * VW],
                xT[ci][:, 128 * st:128 * (st + 1)],
                wv_t[ci][:],
                start=(ci == 0), stop=(ci == 5))
            if ci == 5:
                # drain on DVE so the ACT engine stays dedicated to exp
                nc.vector.tensor_copy(v_t[st][:], t[:, 0:HG * VW])
                nc.gpsimd.memset(
                    v_t[st][:].rearrange("p (g c) -> p g c", c=VW)[:, :, 96],
                    1.0)
                del st_state[key]

        def run_step(job, ci):
            kind = job[0]
            if kind == "q":
                q_step(job[1], job[2], ci)
            elif kind == "k":
                k_step(job[1], job[2], ci)
            else:
                v_step(job[1], ci)

        # ---------------- prologue: minimal start set ----------------
        for ci in range(6):
            k_step(0, 0, ci)     # K0 s-chunk 0
        for jc in range(2):      # Q0 jc 0,1 (item 0 is the B half, l 0..1023)
            for ci in range(6):
                q_step(0, jc, ci)

        # ---------------- attention items ----------------
        # item order: (0,B),(0,A),(1,A),(1,B),(2,A),(3,A),(2,B),(3,B)
        # A = l-tiles 8..15 (the half the peer keeps; l0=1024), B = ours.
        # Item 0 is B so the first exp only needs xT cols 0:1024; the last
        # A item is item 5 so the AllGather overlaps items 6-7.
        items = [(0, 0), (0, 1), (1, 1), (1, 0), (2, 1), (3, 1), (2, 0),
                 (3, 0)]

        # first/second item index per head and half->item map
        first_item = {}
        item_of = {}
        for i, (hh, hf) in enumerate(items):
            first_item.setdefault(hh, i)
            item_of[(hh, hf)] = i

        # global fill jobs with absolute slot deadlines (slot=item*24+tile)
        jobs = []
        jobs.append((("v", 0), 3))
        jobs.append((("v", 1), 4))
        for st in range(2, NST):
            jobs.append((("v", st), max(st - 2, 5)))
        for sc in range(1, 6):
            jobs.append((("k", 0, sc), 4 * sc - 1))
        jobs.append((("q", 0, 2), 24 * item_of[(0, 1)] - 1))
        jobs.append((("q", 0, 3), 24 * item_of[(0, 1)] - 1))
        for h in range(1, HG):
            fi = first_item[h]
            si = item_of[(h, 1 - items[fi][1])]
            for sc in range(6):
                jobs.append((("k", h, sc), 24 * fi + 4 * sc - 1))
            half0 = items[fi][1]
            for jc in ((2, 3) if half0 == 1 else (0, 1)):
                jobs.append((("q", h, jc), 24 * fi - 1))
            for jc in ((0, 1) if half0 == 1 else (2, 3)):
                jobs.append((("q", h, jc), 24 * (si - 1) - 1))
        jobs.sort(key=lambda j: j[1])
        fill_steps = [(job, ci, dl) for job, dl in jobs for ci in range(6)]
        n_fill = len(fill_steps)
        fill_pos = 0

        # PSUM: qk 2x2 banks + pv 2x1 + proj 2x1 = 8 banks.
        # NOTE: concurrent matmul accumulation groups must each own a full
        # PSUM bank (sub-bank column-offset groups corrupt each other on hw),
        # so PV runs per l-tile as a sequential 24-step group over the
        # item's stored exp tiles.
        qk_ps = attn_ctx.enter_context(
            tc.tile_pool(name="qk_ps", bufs=2, space="PSUM"))
        pv_ps = attn_ctx.enter_context(
            tc.tile_pool(name="pv_ps", bufs=2, space="PSUM"))
        ex_pool = attn_ctx.enter_context(tc.tile_pool(name="ex", bufs=26))
        nrm_pool = attn_ctx.enter_context(tc.tile_pool(name="nrm", bufs=4))

        N_SLOTS = len(items) * NST
        for it, (h, half) in enumerate(items):
            l0 = 1024 * half
            exs = []
            for t in range(NST):
                slot = it * NST + t
                # run fill steps: all with deadline <= this slot, plus keep
                # the global pace (even spread across all slots)
                pace = ((slot + 1) * n_fill) // N_SLOTS
                while fill_pos < n_fill and (
                        fill_steps[fill_pos][2] is not None
                        and fill_steps[fill_pos][2] <= slot
                        or fill_pos < pace):
                    job, ci, _ = fill_steps[fill_pos]
                    run_step(job, ci)
                    fill_pos += 1
                qk = qk_ps.tile([128, 1024], F32, tag="qk", name="qk")
                for lw in range(2):   # one PSUM bank (512 f32) per matmul
                    nc.tensor.matmul(
                        qk[:, 512 * lw:512 * (lw + 1)],
                        kT[h][:, 128 * t:128 * (t + 1)],
                        qT[h][:, l0 + 512 * lw:l0 + 512 * (lw + 1)],
                        start=True, stop=True)
                ex = ex_pool.tile([128, 1024], BF16, tag="ex", name="ex")
                nc.scalar.activation(ex[:], qk[:], AF.Exp, scale=SCALE)
                exs.append(ex)

            def pv_step(pvt, lt, t, start, stop):
                vh = v_t[t][:].rearrange("p (g c) -> p g c", c=VW)
                nc.tensor.matmul(
                    pvt, exs[t][:, 128 * lt:128 * (lt + 1)], vh[:, h, :],
                    start=start, stop=stop)

            def normalize(pvt, lt):
                glt = 8 * half + lt
                rec = nrm_pool.tile([128, 1], F32, tag="rec", name="rec")
                nc.vector.reciprocal(rec[:], pvt[:, 96:97])
                nc.vector.tensor_scalar_mul(
                    ao_t[glt][:, D * h:D * (h + 1)], pvt[:, 0:96], rec[:])

            for lt in range(8):
                pv = pv_ps.tile([128, VW], F32, tag="pv", name="pv")
                for t in range(NST):
                    pv_step(pv[:], lt, t, t == 0, t == NST - 1)
                normalize(pv[:], lt)

            if half == 1:
                # this head's peer-half ao is complete: fire its gather now
                exchange_head(h)

        # close attention-phase PSUM pools before the epilogue opens its own
        attn_ctx.close()

        # ---------------- aoT via DMA xbar transposes ----------------
        aoT_pool = ctx.enter_context(tc.tile_pool(name="aoT", bufs=1))
        # one [128, 3, 1024] tensor per source (own, chunk0, chunk1)
        aoT = [aoT_pool.tile([128, 3 * LLOC], BF16, tag=f"aoT{s}",
                             name=f"aoT{s}") for s in range(3)]
        for lt in range(8):
            nc.sync.dma_start_transpose(
                aoT[0][:].rearrange("p (c l) -> p c l", c=3)
                [:, :, 128 * lt:128 * (lt + 1)],
                ao_t[lt][:])
            for r in range(2):
                nc.sync.dma_start_transpose(
                    aoT[1 + r][:].rearrange("p (c l) -> p c l", c=3)
                    [:, :, 128 * lt:128 * (lt + 1)],
                    recv[r][:, CG * lt:CG * (lt + 1)])

        # ---------------- O proj + LayerNorm ----------------
        with tc.tile_pool(name="ln", bufs=3) as ln_pool, \
             tc.tile_pool(name="wo_ps", bufs=3, space="PSUM") as wo_ps:
            for lt in range(8):
                wp = wo_ps.tile([128, C + 1], F32, tag="wop", name="wop")
                for n0, n1 in ((0, 512), (512, C + 1)):
                    for k in range(9):
                        src, ci = divmod(k, 3)
                        nc.tensor.matmul(
                            wp[:, n0:n1],
                            aoT[src][:].rearrange("p (c l) -> p c l", c=3)
                            [:, ci, 128 * lt:128 * (lt + 1)],
                            wo_t[k][:, n0:n1], start=(k == 0), stop=(k == 8))
                sq = ln_pool.tile([128, C], F32, tag="sq")
                s2 = ln_pool.tile([128, 1], F32, tag="s2")
                nc.scalar.activation(sq[:], wp[:, 0:C], AF.Square,
                                     accum_out=s2[:])
                negmu = ln_pool.tile([128, 1], F32, tag="negmu")
                nc.vector.tensor_scalar_mul(negmu[:], wp[:, C:C + 1], -1.0 / C)
                mu2 = ln_pool.tile([128, 1], F32, tag="mu2")
                nc.vector.tensor_mul(mu2[:], negmu[:], negmu[:])
                veps = ln_pool.tile([128, 1], F32, tag="veps")
                nc.vector.tensor_scalar(
                    veps[:], s2[:], 1.0 / C, EPS,
                    op0=mybir.AluOpType.mult, op1=mybir.AluOpType.add)
                nc.vector.tensor_sub(veps[:], veps[:], mu2[:])
                std = ln_pool.tile([128, 1], F32, tag="std")
                nc.scalar.activation(std[:], veps[:], AF.Sqrt)
                rstd = ln_pool.tile([128, 1], F32, tag="rstd")
                nc.vector.reciprocal(rstd[:], std[:])
                t1 = ln_pool.tile([128, C], F32, tag="t1")
                nc.vector.scalar_tensor_tensor(
                    t1[:], wp[:, 0:C], negmu[:], gamma_t[:],
                    op0=mybir.AluOpType.add, op1=mybir.AluOpType.mult)
                ot = ln_pool.tile([128, C], BF16, tag="ot")
                nc.vector.scalar_tensor_tensor(
                    ot[:], t1[:], rstd[:], beta_t[:],
                    op0=mybir.AluOpType.mult, op1=mybir.AluOpType.add)
                nc.sync.dma_start(out_ext[128 * lt:128 * (lt + 1), :], ot[:])

    nc.finalize()
    return nc


_CACHE = {}


def _get_nc():
    if "nc" not in _CACHE:
        _CACHE["nc"] = build_nc()
    return _CACHE["nc"]


def make_in_maps(h, vis, Wq, Wk, Wv, Wo, ln_gamma, ln_beta):
    import ml_dtypes
    bf16 = ml_dtypes.bfloat16
    h = np.asarray(h, np.float32)
    vis = np.asarray(vis, np.float32)
    WqT = np.asarray(Wq, np.float32).T    # [C, C] cols = output dim
    WkT = np.asarray(Wk, np.float32).T
    WvT = np.asarray(Wv, np.float32).T
    WoT = np.asarray(Wo, np.float32).T    # [C(in rows), C(out cols)]
    gmb = np.ascontiguousarray(
        np.tile(np.asarray(ln_gamma, np.float32)[None, :], (128, 1)))
    btb = np.ascontiguousarray(
        np.tile(np.asarray(ln_beta, np.float32)[None, :], (128, 1)))

    in_maps = []
    for core in range(N_CORES):
        b, hg = core // 2, core % 2
        roll = 1024 * hg
        h_r = np.roll(h[b], -roll, axis=0)           # local row j = global
        x_r = np.concatenate([h_r, vis[b]], axis=0)  # (roll + j) % 2048
        xt = np.ascontiguousarray(x_r.T.astype(bf16))
        c0 = CG * hg
        wq = np.ascontiguousarray(WqT[:, c0:c0 + CG].astype(bf16))
        wk = np.ascontiguousarray(WkT[:, c0:c0 + CG].astype(bf16))
        wv_blk = WvT[:, c0:c0 + CG].reshape(C, HG, D)
        wv = np.zeros((C, HG, VW), np.float32)
        wv[:, :, 0:D] = wv_blk
        wv = np.ascontiguousarray(wv.reshape(C, HG * VW).astype(bf16))
        # wo rows: own 384 (my heads), chunk0 (= heads 0..3 if peer is rank0
        # else zeros), chunk1 (= heads 4..7 if peer is rank1 else zeros)
        wo = np.zeros((3 * CG, C + 1), np.float32)
        own_rows = WoT[c0:c0 + CG, :]                # my heads' input rows
        wo[0:CG, 0:C] = own_rows
        peer = 1 - hg
        pc0 = CG * peer
        wo[CG * (1 + peer):CG * (2 + peer), 0:C] = WoT[pc0:pc0 + CG, :]
        wo[:, C] = wo[:, 0:C].sum(axis=1)
        wo = np.ascontiguousarray(wo.astype(bf16))
        in_maps.append({
            "xT": xt, "wq": wq, "wk": wk, "wv": wv, "wo": wo,
            "gammab": gmb, "betab": btb,
        })
    return in_maps


def run(in_maps, trace=False, **kw):
    nc = _get_nc()
    return run_bass_kernel_spmd(nc, in_maps, core_ids=list(range(N_CORES)),
                                trace=trace, **kw)


def assemble(results):
    full = np.empty((B, L, C), np.float32)
    for core in range(N_CORES):
        b, hg = core // 2, core % 2
        full[b, 1024 * hg:1024 * (hg + 1)] = results[core]["out"]
    return full


def kernel(h, vis, Wq, Wk, Wv, Wo, ln_gamma, ln_beta):
    in_maps = make_in_maps(h, vis, Wq, Wk, Wv, Wo, ln_gamma, ln_beta)
    res = run(in_maps, trace=False)
    return assemble(res.results)


# revision 57
# speedup vs baseline: 1.0769x; 1.0769x over previous
"""Trainium2 Bass kernel for nn_AdapterBlock (cross-attention adapter block).

Reference (per batch b):
    x   = concat(h[b], vis[b])                    # [S=3072, C=768]
    q   = h @ Wq.T ; k = x @ Wk.T ; v = x @ Wv.T  # H=8 heads, D=96
    att = softmax(q k^T / sqrt(D)) v              # [L, C]
    out = LayerNorm(att @ Wo.T) * gamma + beta

Sharding: 8 cores = (batch b = core//2) x (head-group hg = core%2).
Each core projects K/V/Q only for its own 4 heads (so the pair shares the
projection work -- no duplication), attends over the full L=2048 query
rows for those heads, then the pair exchanges attention outputs via an
AllGather (pairs replica groups) so each core can out-project + LayerNorm
its own half of the query rows with the full 768-dim contraction.

All cross-core asymmetry is resolved host-side so the SPMD program is
static:
  * the h-token rows are rolled per core so "rows this core keeps" are
    always local rows 0..1023 (l-tiles 0-7) and "rows the peer keeps" are
    local l-tiles 8-15 (kv-token order doesn't matter for attention);
  * the gathered buffer has one chunk per pair rank; the out-projection
    contracts over own ao + both gathered chunks, with the Wo rows for
    this core's own chunk zeroed host-side (its data covers the wrong
    rows) and the peer chunk carrying the peer heads' Wo rows.

Orientation choices (the cost model charges a matmul out_free x steps):
  * Q/K projected direct-transposed (qT/kT [96, L/S]) for attention.
  * V natural [s, 97/head] with a ones column => softmax denominator for
    free in the PV matmul's 97th output column.
  * scores [s-tile 128, l 1024]: one matmul per s-tile; exp on ACT.
  * PV natural [l-tile 128, 97] accumulating over 24 s-tiles (lhsT = exp
    tile slice, rhs = v tile): full 128-partition packing.
  * ao -> aoT via DMA xbar transposes (no PE time).
  * O natural [l-tile, 769] with a host row-sums column = LN mean free.
Q/K projections for head h+1 and the V projections are interleaved into
the ACT(exp)-bound attention items with an EDF scheduler. The v
PSUM->SBUF drains run on DVE (not ACT) so the exp stream never waits
behind them, and the xT / weight loads are batched into one strided DMA
per wave (HWDGE issue is ~625ns per instruction and serializes the
prologue).
"""

from contextlib import ExitStack

import numpy as np

import concourse.bacc as bacc
import concourse.mybir as mybir
import concourse.tile as tile
from concourse.bass_utils import run_bass_kernel_spmd

F32 = mybir.dt.float32
BF16 = mybir.dt.bfloat16
AF = mybir.ActivationFunctionType

B, L, NV, C = 4, 2048, 1024, 768
H, D = 8, 96
S = L + NV             # 3072 kv tokens
SCALE = float(D) ** -0.5
N_CORES = 8
NST = S // 128         # 24 s-tiles
EPS = 1e-5
HG = 4                 # heads per core
CG = HG * D            # 384 feature cols per core
VW = 97                # v cols per head (96 + denominator ones col)
LLOC = L // 2          # 1024 query rows kept per core
REP_GROUPS = [[0, 1], [2, 3], [4, 5], [6, 7]]


def build_nc():
    nc = bacc.Bacc("TRN2", target_bir_lowering=False, num_devices=N_CORES)
    xt_in = nc.declare_dram_parameter("xT", [C, S], BF16, isOutput=False)
    wq_in = nc.declare_dram_parameter("wq", [C, CG], BF16, isOutput=False)
    wk_in = nc.declare_dram_parameter("wk", [C, CG], BF16, isOutput=False)
    wv_in = nc.declare_dram_parameter("wv", [C, HG * VW], BF16, isOutput=False)
    # wo rows: [own-ao 384 | gathered-chunk0 384 | gathered-chunk1 384]
    wo_in = nc.declare_dram_parameter("wo", [3 * CG, C + 1], BF16,
                                      isOutput=False)
    gmb_in = nc.declare_dram_parameter("gammab", [128, C], F32, isOutput=False)
    btb_in = nc.declare_dram_parameter("betab", [128, C], F32, isOutput=False)
    out_ext = nc.declare_dram_parameter("out", [LLOC, C], BF16, isOutput=True)

    with tile.TileContext(nc) as tc, ExitStack() as ctx:
        # ---------------- persistent tensors ----------------
        xT_pool = ctx.enter_context(tc.tile_pool(name="xT", bufs=1))
        xT_all = xT_pool.tile([128, 6 * S], BF16, tag="xT", name="xT")
        xT = [xT_all[:].rearrange("p (c s) -> p c s", c=6)[:, ci, :]
              for ci in range(6)]
        w_pool = ctx.enter_context(tc.tile_pool(name="w", bufs=1))
        wq_all = w_pool.tile([128, 6 * CG], BF16, tag="wq", name="wq")
        wk_all = w_pool.tile([128, 6 * CG], BF16, tag="wk", name="wk")
        wv_all = w_pool.tile([128, 6 * HG * VW], BF16, tag="wv", name="wv")
        wq_t = [wq_all[:, CG * ci:CG * (ci + 1)] for ci in range(6)]
        wk_t = [wk_all[:, CG * ci:CG * (ci + 1)] for ci in range(6)]
        wv_t = [wv_all[:, HG * VW * ci:HG * VW * (ci + 1)]
                for ci in range(6)]

        kq_pool = ctx.enter_context(tc.tile_pool(name="kq", bufs=1))
        kT = [kq_pool.tile([96, S], BF16, tag=f"kT{h}", name=f"kT{h}")
              for h in range(HG)]
        qT = [kq_pool.tile([96, L], BF16, tag=f"qT{h}", name=f"qT{h}")
              for h in range(HG)]
        v_pool = ctx.enter_context(tc.tile_pool(name="v", bufs=1))
        v_t = [v_pool.tile([128, HG * VW], BF16, tag=f"v{t}", name=f"v{t}")
               for t in range(NST)]
        ao_pool = ctx.enter_context(tc.tile_pool(name="ao", bufs=1))
        # own 4 heads' normalized attention out, all 16 l-tiles, 384 cols
        ao_t = [ao_pool.tile([128, CG], BF16, tag=f"ao{lt}", name=f"ao{lt}")
                for lt in range(16)]

        const_pool = ctx.enter_context(tc.tile_pool(name="cst", bufs=1))
        gamma_t = const_pool.tile([128, C], F32, tag="gamma")
        beta_t = const_pool.tile([128, C], F32, tag="beta")

        # stream xT in batched strided waves: cols 0:1024 first (covers
        # the prologue's K0/Q0/V needs); the rest follows in two waves.
        xt_src = xt_in[:].rearrange("(c p) s -> p c s", p=128)
        xT_v = xT_all[:].rearrange("p (c s) -> p c s", c=6)
        wk_v = wk_all[:].rearrange("p (c g) -> p c g", c=6)
        wk_src = wk_in[:].rearrange("(c p) g -> p c g", p=128)
        wq_v = wq_all[:].rearrange("p (c g) -> p c g", c=6)
        wq_src = wq_in[:].rearrange("(c p) g -> p c g", p=128)
        nc.sync.dma_start(wk_v[:, 0:3], wk_src[:, 0:3])
        nc.sync.dma_start(xT_v[:, 0:3, 0:512], xt_src[:, 0:3, 0:512])
        nc.sync.dma_start(wk_v[:, 3:6], wk_src[:, 3:6])
        nc.sync.dma_start(xT_v[:, 3:6, 0:512], xt_src[:, 3:6, 0:512])
        nc.sync.dma_start(wq_v[:, 0:3], wq_src[:, 0:3])
        nc.sync.dma_start(xT_v[:, 0:3, 512:1024], xt_src[:, 0:3, 512:1024])
        nc.sync.dma_start(wq_v[:, 3:6], wq_src[:, 3:6])
        nc.sync.dma_start(xT_v[:, 3:6, 512:1024], xt_src[:, 3:6, 512:1024])
        nc.sync.dma_start(wv_all[:].rearrange("p (c g) -> p c g", c=6),
                          wv_in[:].rearrange("(c p) g -> p c g", p=128))
        nc.sync.dma_start(xT_v[:, :, 1024:2048], xt_src[:, :, 1024:2048])
        nc.sync.dma_start(xT_v[:, :, 2048:S], xt_src[:, :, 2048:S])
        nc.sync.dma_start(gamma_t[:], gmb_in[:])
        nc.sync.dma_start(beta_t[:], btb_in[:])
        # Wo is persistent and loaded early so the epilogue never waits on it
        wo_pool = ctx.enter_context(tc.tile_pool(name="wo", bufs=1))
        wo_t = [wo_pool.tile([128, C + 1], BF16, tag=f"wo{k}", name=f"wo{k}")
                for k in range(9)]
        for k in range(9):
            nc.sync.dma_start(wo_t[k][:], wo_in[128 * k:128 * (k + 1), :])

        # exchange buffers: per-head AllGather (pairs), pipelined so each
        # gather overlaps the remaining attention items
        dram = ctx.enter_context(tc.tile_pool(name="dram", bufs=1,
                                              space="DRAM"))
        ag_in = [dram.tile([128, 8 * D], BF16, tag=f"agi{hh}",
                           name=f"agi{hh}") for hh in range(HG)]
        ag_out = [dram.tile([256, 8 * D], BF16, tag=f"ago{hh}",
                            name=f"ago{hh}") for hh in range(HG)]
        recv_pool = ctx.enter_context(tc.tile_pool(name="recv", bufs=1))
        recv = [recv_pool.tile([128, 8 * CG], BF16, tag=f"rc{r}",
                               name=f"rc{r}") for r in range(2)]
        # transposed ao sources for the out-projection (own, chunk0,
        # chunk1). Allocated here -- below the attention-phase pools on
        # the SBUF stack and outliving them -- in the space freed by the
        # smaller quarter-sweep exp pool; the transposes are issued inside
        # the item loop so their ~650ns-per-instruction HWDGE
        # serialization overlaps attention instead of the epilogue.
        aoT_pool = ctx.enter_context(tc.tile_pool(name="aoT", bufs=1))
        aoT = [aoT_pool.tile([128, 3 * LLOC], BF16, tag=f"aoT{s}",
                             name=f"aoT{s}") for s in range(3)]

        def exchange_head(hh):
            for lt in range(8):
                nc.gpsimd.dma_start(ag_in[hh][:, D * lt:D * (lt + 1)],
                                    ao_t[8 + lt][:, D * hh:D * (hh + 1)])
            nc.gpsimd.collective_compute(
                "AllGather", mybir.AluOpType.bypass,
                replica_groups=REP_GROUPS,
                ins=[ag_in[hh][:]], outs=[ag_out[hh][:]])
            for r in range(2):
                nc.sync.dma_start(
                    recv[r][:].rearrange("p (l c) -> p l c", c=CG)
                    [:, :, D * hh:D * (hh + 1)],
                    ag_out[hh][128 * r:128 * (r + 1), :]
                    .rearrange("p (l c) -> p l c", c=D))

        # ---------------- projection step helpers ----------------
        attn_ctx = ExitStack()
        proj_ps = attn_ctx.enter_context(
            tc.tile_pool(name="proj_ps", bufs=2, space="PSUM"))
        st_state = {}

        def q_step(h, jc, ci):
            key = ("q", h, jc)
            if ci == 0:
                st_state[key] = proj_ps.tile([128, 512], F32, tag="proj",
                                             name="prps")
            t = st_state[key]
            nc.tensor.matmul(
                t[0:96, :], wq_t[ci][:, D * h:D * (h + 1)],
                xT[ci][:, 512 * jc:512 * (jc + 1)],
                start=(ci == 0), stop=(ci == 5))
            if ci == 5:
                nc.vector.tensor_copy(
                    qT[h][:, 512 * jc:512 * (jc + 1)], t[0:96, :])
                del st_state[key]

        def k_step(h, sc, ci):
            key = ("k", h, sc)
            if ci == 0:
                st_state[key] = proj_ps.tile([128, 512], F32, tag="proj",
                                             name="prps")
            t = st_state[key]
            nc.tensor.matmul(
                t[0:96, :], wk_t[ci][:, D * h:D * (h + 1)],
                xT[ci][:, 512 * sc:512 * (sc + 1)],
                start=(ci == 0), stop=(ci == 5))
            if ci == 5:
                nc.vector.tensor_copy(
                    kT[h][:, 512 * sc:512 * (sc + 1)], t[0:96, :])
                del st_state[key]

        def v_step(st, ci):
            key = ("v", st)
            if ci == 0:
                st_state[key] = proj_ps.tile([128, 512], F32, tag="proj",
                                             name="prps")
            t = st_state[key]
            nc.tensor.matmul(
                t[:, 0:HG * VW],
                xT[ci][:, 128 * st:128 * (st + 1)],
                wv_t[ci][:],
                start=(ci == 0), stop=(ci == 5))
            if ci == 5:
                # drain on DVE so the ACT engine stays dedicated to exp
                nc.vector.tensor_copy(v_t[st][:], t[:, 0:HG * VW])
                nc.gpsimd.memset(
                    v_t[st][:].rearrange("p (g c) -> p g c", c=VW)[:, :, 96],
                    1.0)
                del st_state[key]

        def run_step(job, ci):
            kind = job[0]
            if kind == "q":
                q_step(job[1], job[2], ci)
            elif kind == "k":
                k_step(job[1], job[2], ci)
            else:
                v_step(job[1], ci)

        # ---------------- prologue: minimal start set ----------------
        for ci in range(6):
            k_step(0, 0, ci)     # K0 s-chunk 0
        for jc in range(2):      # Q0 jc 0,1 (item 0 is the B half, l 0..1023)
            for ci in range(6):
                q_step(0, jc, ci)

        # ---------------- attention items ----------------
        # item order: (0,B),(0,A),(1,A),(1,B),(2,A),(3,A),(2,B),(3,B)
        # A = l-tiles 8..15 (the half the peer keeps; l0=1024), B = ours.
        # Item 0 is B so the first exp only needs xT cols 0:1024; the last
        # A item is item 5 so the AllGather overlaps items 6-7.
        items = [(0, 0), (0, 1), (1, 1), (1, 0), (2, 1), (3, 1), (2, 0),
                 (3, 0)]

        # first/second item index per head and half->item map
        first_item = {}
        item_of = {}
        for i, (hh, hf) in enumerate(items):
            first_item.setdefault(hh, i)
            item_of[(hh, hf)] = i

        # global fill jobs with absolute slot deadlines (slot=item*24+tile)
        jobs = []
        jobs.append((("v", 0), 3))
        jobs.append((("v", 1), 4))
        for st in range(2, NST):
            jobs.append((("v", st), max(st - 2, 5)))
        for sc in range(1, 6):
            jobs.append((("k", 0, sc), 4 * sc - 1))
        jobs.append((("q", 0, 2), 24 * item_of[(0, 1)] - 1))
        jobs.append((("q", 0, 3), 24 * item_of[(0, 1)] - 1))
        for h in range(1, HG):
            fi = first_item[h]
            si = item_of[(h, 1 - items[fi][1])]
            for sc in range(6):
                jobs.append((("k", h, sc), 24 * fi + 4 * sc - 1))
            half0 = items[fi][1]
            for jc in ((2, 3) if half0 == 1 else (0, 1)):
                jobs.append((("q", h, jc), 24 * fi - 1))
            for jc in ((0, 1) if half0 == 1 else (2, 3)):
                jobs.append((("q", h, jc), 24 * (si - 1) - 1))
        jobs.sort(key=lambda j: j[1])
        fill_steps = [(job, ci, dl) for job, dl in jobs for ci in range(6)]
        n_fill = len(fill_steps)
        fill_pos = 0

        # PSUM: qk 2x2 banks + pv 2x1 + proj 2x1 = 8 banks.
        # NOTE: concurrent matmul accumulation groups must each own a full
        # PSUM bank (sub-bank column-offset groups corrupt each other on hw),
        # so PV runs per l-tile as a sequential 24-step group over the
        # item's stored exp tiles.
        qk_ps = attn_ctx.enter_context(
            tc.tile_pool(name="qk_ps", bufs=2, space="PSUM"))
        pv_ps = attn_ctx.enter_context(
            tc.tile_pool(name="pv_ps", bufs=2, space="PSUM"))
        # PV runs as four 6-s-tile quarter-sweeps accumulated into an f32
        # SBUF tile on DVE, so only 6 exp tiles are pinned at a time and
        # the ~7 spare buffers let the ACT exp stream run ahead through
        # each PV phase instead of stalling ~5.7us per item.
        ex_pool = attn_ctx.enter_context(tc.tile_pool(name="ex", bufs=13))
        acc_pool = attn_ctx.enter_context(tc.tile_pool(name="acc", bufs=8))
        nrm_pool = attn_ctx.enter_context(tc.tile_pool(name="nrm", bufs=4))

        QT = NST // 4
        N_SLOTS = len(items) * NST
        for it, (h, half) in enumerate(items):
            l0 = 1024 * half
            acc = []

            def pv_step(pvt, ex, lt, vt, start, stop):
                vh = v_t[vt][:].rearrange("p (g c) -> p g c", c=VW)
                nc.tensor.matmul(
                    pvt, ex[:, 128 * lt:128 * (lt + 1)], vh[:, h, :],
                    start=start, stop=stop)

            for ph in range(4):
                exs = []
                for tt in range(QT):
                    t = QT * ph + tt
                    slot = it * NST + t
                    # run fill steps: all with deadline <= this slot, plus
                    # keep the global pace (even spread across all slots)
                    pace = ((slot + 1) * n_fill) // N_SLOTS
                    while fill_pos < n_fill and (
                            fill_steps[fill_pos][2] is not None
                            and fill_steps[fill_pos][2] <= slot
                            or fill_pos < pace):
                        job, ci, _ = fill_steps[fill_pos]
                        run_step(job, ci)
                        fill_pos += 1
                    qk = qk_ps.tile([128, 1024], F32, tag="qk", name="qk")
                    for lw in range(2):   # one PSUM bank per matmul
                        nc.tensor.matmul(
                            qk[:, 512 * lw:512 * (lw + 1)],
                            kT[h][:, 128 * t:128 * (t + 1)],
                            qT[h][:, l0 + 512 * lw:l0 + 512 * (lw + 1)],
                            start=True, stop=True)
                    ex = ex_pool.tile([128, 1024], BF16, tag="ex", name="ex")
                    nc.scalar.activation(ex[:], qk[:], AF.Exp, scale=SCALE)
                    exs.append(ex)

                for lt in range(8):
                    pv = pv_ps.tile([128, VW], F32, tag="pv", name="pv")
                    for tt in range(QT):
                        pv_step(pv[:], exs[tt], lt, QT * ph + tt,
                                tt == 0, tt == QT - 1)
                    if ph == 0:
                        pa = acc_pool.tile([128, VW], F32, tag="acc",
                                           name="acc")
                        nc.vector.tensor_copy(pa[:], pv[:])
                        acc.append(pa)
                        continue
                    nc.vector.tensor_tensor(acc[lt][:], acc[lt][:], pv[:],
                                            mybir.AluOpType.add)
                    if ph < 3:
                        continue
                    glt = 8 * half + lt
                    rec = nrm_pool.tile([128, 1], F32, tag="rec", name="rec")
                    nc.vector.reciprocal(rec[:], acc[lt][:, 96:97])
                    nc.vector.tensor_scalar_mul(
                        ao_t[glt][:, D * h:D * (h + 1)], acc[lt][:, 0:96],
                        rec[:])
                    if it == 7:
                        # own ao for this l-tile is final: transpose it now
                        nc.sync.dma_start_transpose(
                            aoT[0][:].rearrange("p (c l) -> p c l", c=3)
                            [:, :, 128 * lt:128 * (lt + 1)],
                            ao_t[lt][:])

            if half == 1:
                # this head's peer-half ao is complete: fire its gather now
                exchange_head(h)
                if h == 3:
                    # last gather: transpose the gathered chunks as soon
                    # as their recv DMAs land (SP parks until then; it has
                    # no other work before the epilogue)
                    for lt in range(8):
                        for r in range(2):
                            nc.sync.dma_start_transpose(
                                aoT[1 + r][:].rearrange(
                                    "p (c l) -> p c l", c=3)
                                [:, :, 128 * lt:128 * (lt + 1)],
                                recv[r][:, CG * lt:CG * (lt + 1)])

        # close attention-phase PSUM pools before the epilogue opens its own
        attn_ctx.close()

        # ---------------- O proj + LayerNorm ----------------
        with tc.tile_pool(name="ln", bufs=3) as ln_pool, \
             tc.tile_pool(name="wo_ps", bufs=3, space="PSUM") as wo_ps:
            for lt in range(8):
                wp = wo_ps.tile([128, C + 1], F32, tag="wop", name="wop")
                for n0, n1 in ((0, 512), (512, C + 1)):
                    for k in range(9):
                        src, ci = divmod(k, 3)
                        nc.tensor.matmul(
                            wp[:, n0:n1],
                            aoT[src][:].rearrange("p (c l) -> p c l", c=3)
                            [:, ci, 128 * lt:128 * (lt + 1)],
                            wo_t[k][:, n0:n1], start=(k == 0), stop=(k == 8))
                sq = ln_pool.tile([128, C], F32, tag="sq")
                s2 = ln_pool.tile([128, 1], F32, tag="s2")
                nc.scalar.activation(sq[:], wp[:, 0:C], AF.Square,
                                     accum_out=s2[:])
                negmu = ln_pool.tile([128, 1], F32, tag="negmu")
                nc.vector.tensor_scalar_mul(negmu[:], wp[:, C:C + 1], -1.0 / C)
                mu2 = ln_pool.tile([128, 1], F32, tag="mu2")
                nc.vector.tensor_mul(mu2[:], negmu[:], negmu[:])
                veps = ln_pool.tile([128, 1], F32, tag="veps")
                nc.vector.tensor_scalar(
                    veps[:], s2[:], 1.0 / C, EPS,
                    op0=mybir.AluOpType.mult, op1=mybir.AluOpType.add)
                nc.vector.tensor_sub(veps[:], veps[:], mu2[:])
                std = ln_pool.tile([128, 1], F32, tag="std")
                nc.scalar.activation(std[:], veps[:], AF.Sqrt)
                rstd = ln_pool.tile([128, 1], F32, tag="rstd")
                nc.vector.reciprocal(rstd[:], std[:])
                t1 = ln_pool.tile([128, C], F32, tag="t1")
                nc.vector.scalar_tensor_tensor(
                    t1[:], wp[:, 0:C], negmu[:], gamma_t[:],
                    op0=mybir.AluOpType.add, op1=mybir.AluOpType.mult)
                ot = ln_pool.tile([128, C], BF16, tag="ot")
                nc.vector.scalar_tensor_tensor(
                    ot[:], t1[:], rstd[:], beta_t[:],
                    op0=mybir.AluOpType.mult, op1=mybir.AluOpType.add)
                nc.sync.dma_start(out_ext[128 * lt:128 * (lt + 1), :], ot[:])

    nc.finalize()
    return nc


_CACHE = {}


def _get_nc():
    if "nc" not in _CACHE:
        _CACHE["nc"] = build_nc()
    return _CACHE["nc"]


def make_in_maps(h, vis, Wq, Wk, Wv, Wo, ln_gamma, ln_beta):
    import ml_dtypes
    bf16 = ml_dtypes.bfloat16
    h = np.asarray(h, np.float32)
    vis = np.asarray(vis, np.float32)
    WqT = np.asarray(Wq, np.float32).T    # [C, C] cols = output dim
    WkT = np.asarray(Wk, np.float32).T
    WvT = np.asarray(Wv, np.float32).T
    WoT = np.asarray(Wo, np.float32).T    # [C(in rows), C(out cols)]
    gmb = np.ascontiguousarray(
        np.tile(np.asarray(ln_gamma, np.float32)[None, :], (128, 1)))
    btb = np.ascontiguousarray(
        np.tile(np.asarray(ln_beta, np.float32)[None, :], (128, 1)))

    in_maps = []
    for core in range(N_CORES):
        b, hg = core // 2, core % 2
        roll = 1024 * hg
        h_r = np.roll(h[b], -roll, axis=0)           # local row j = global
        x_r = np.concatenate([h_r, vis[b]], axis=0)  # (roll + j) % 2048
        xt = np.ascontiguousarray(x_r.T.astype(bf16))
        c0 = CG * hg
        wq = np.ascontiguousarray(WqT[:, c0:c0 + CG].astype(bf16))
        wk = np.ascontiguousarray(WkT[:, c0:c0 + CG].astype(bf16))
        wv_blk = WvT[:, c0:c0 + CG].reshape(C, HG, D)
        wv = np.zeros((C, HG, VW), np.float32)
        wv[:, :, 0:D] = wv_blk
        wv = np.ascontiguousarray(wv.reshape(C, HG * VW).astype(bf16))
        # wo rows: own 384 (my heads), chunk0 (= heads 0..3 if peer is rank0
        # else zeros), chunk1 (= heads 4..7 if peer is rank1 else zeros)
        wo = np.zeros((3 * CG, C + 1), np.float32)
        own_rows = WoT[c0:c0 + CG, :]                # my heads' input rows
        wo[0:CG, 0:C] = own_rows
        peer = 1 - hg
        pc0 = CG * peer
        wo[CG * (1 + peer):CG * (2 + peer), 0:C] = WoT[pc0:pc0 + CG, :]
        wo[:, C] = wo[:, 0:C].sum(axis=1)
        wo = np.ascontiguousarray(wo.astype(bf16))
        in_maps.append({
            "xT": xt, "wq": wq, "wk": wk, "wv": wv, "wo": wo,
            "gammab": gmb, "betab": btb,
        })
    return in_maps


def run(in_maps, trace=False, **kw):
    nc = _get_nc()
    return run_bass_kernel_spmd(nc, in_maps, core_ids=list(range(N_CORES)),
                                trace=trace, **kw)


def assemble(results):
    full = np.empty((B, L, C), np.float32)
    for core in range(N_CORES):
        b, hg = core // 2, core % 2
        full[b, 1024 * hg:1024 * (hg + 1)] = results[core]["out"]
    return full


def kernel(h, vis, Wq, Wk, Wv, Wo, ln_gamma, ln_beta):
    in_maps = make_in_maps(h, vis, Wq, Wk, Wv, Wo, ln_gamma, ln_beta)
    res = run(in_maps, trace=False)
    return assemble(res.results)
